# revision 36
# baseline (speedup 1.0000x reference)
"""AttentionWithRoPE distributed Trainium2 kernel (8 NeuronCores).

Sharding: pure 8-way tensor parallel over heads (2 heads = 128 hidden cols
per core), both batches on every core (seq concatenated to 4096 cols).
Everything stays transposed ([feature, seq] layouts) so no on-device
transposes are needed anywhere:
  - QKV projections consume xT (host-transposed, batch-concat, bf16) as the
    moving operand, streamed in [128, 512] chunks.
  - RoPE applied on packed qT/kT [128, s] tiles (head0 rows 0-63, head1 rows
    64-127); the 32-row half-rotation is done with sbuf->sbuf DMAs, with a
    sign-folded sin table so no negation op is needed; all ops bf16.
  - scores: 64-row PE array tiling -- head0 on tile (0,0), head1 on (64,0),
    K=64 each, running concurrently in the two array halves.
  - exp split across ScalarE (native Exp) and VectorE (custom 2-pass DVE op
    computing (1+x/16384)^16384, rel err <=0.11% for |x|<=6).
  - ctx matmuls split per 64-row kpos half into 4 PSUM accumulators (merged
    on DVE); v tiles carry a ones-column per head so the softmax denominator
    falls out of the matmul at output partition 64.
  - normalization: reciprocal on DVE, partition_broadcast + multiply on
    GpSimd (all-bf16), shipped straight to the AllToAll buffer.
  - AllToAll (bf16, all 8 cores) exchanges 512-row blocks of ctx^T.
  - Output projection with full Wo produces out^T [1024, 512] for this
    core's 512 global rows; host transposes back (free).
Bias folds (host side): v-bias folds into the output bias exactly (softmax
rows sum to 1); q is pre-scaled by 1/sqrt(64) inside its bias-copy.
Compute dtype bf16 (fp32 PSUM accumulation).
"""

import numpy as np

HID = 1024
S = 2048
SB = 2 * S       # both batches, seq-concatenated
NHEAD = 16
D = 64
HPC = 2          # heads per core
OSL = 128        # hidden slice per core (HPC * D)
RB = 512         # global row block per core after AllToAll
NC = 8
ROPE_BASE = 10000.0

EXP_N = 16384.0  # (1+x/N)^N exp approx on DVE
DVE_KS = (2, 5, 8, 11)   # ks tiles exp'd on DVE instead of ScalarE
DUM = 130        # dummy matmuls keeping the PE p-state up across the a2a

_cached = None
_last_in_maps = None


def _register_exp_ops():
    from concourse import dve_ops as DO
    from concourse.dve_spec import (
        Spec, Src0, C0, One, sq, lower, _has_src1 as has_src1)
    from concourse.dve_uop import DveOpSpec
    from concourse.dve_table_gen import dve_ver_for

    have = {op.name: op for op in DO.OPS}
    if "EXP_POW_A" in have:
        return have["EXP_POW_A"], have["EXP_POW_B"]

    b = Src0 * C0 + One
    for _ in range(6):
        b = sq(b)
    specA = Spec(body=b, reference=lambda in0, in1, s0, s1, imm2:
                 (1.0 + in0 * s0) ** 64)
    c = Src0
    for _ in range(8):
        c = sq(c)
    specB = Spec(body=c, reference=lambda in0, in1, s0, s1, imm2: in0 ** 256)

    out = []
    for name, spec in (("EXP_POW_A", specA), ("EXP_POW_B", specB)):
        DO.OPS.append(DO.DveOp(name, spec, subdim=False, uops_sha={}))
        opcode = DO._CUSTOM_DVE_ROW_BASE + len(DO.OPS) - 1
        DO._SUB_OPCODE_FOR_NAME[name] = opcode
        shas = {}
        for ver in ("v3", "v4"):
            try:
                s = DveOpSpec(name=name, opcode=opcode,
                              uops=lower(spec, ver=ver),
                              rd1_en=has_src1(spec))
                shas[ver] = s.sha(ver)
            except Exception:
                pass
        op = DO.DveOp(name, spec, subdim=False, uops_sha=shas)
        DO.OPS[-1] = op
        DO.CUSTOM_DVE_SPECS[name] = spec
        out.append(op)
    return out[0], out[1]


def _build_nc():
    import concourse.bacc as bacc
    import concourse.mybir as mybir
    from concourse import tile

    EXP_A, EXP_B = _register_exp_ops()

    f32 = mybir.dt.float32
    bf16 = mybir.dt.bfloat16
    AF = mybir.ActivationFunctionType

    nc = bacc.Bacc(None, target_bir_lowering=False)

    xT = nc.declare_dram_parameter("xT", [HID, SB], bf16, isOutput=False)
    wqP = nc.declare_dram_parameter("wqP", [128, HID], bf16, isOutput=False)
    wkP = nc.declare_dram_parameter("wkP", [128, HID], bf16, isOutput=False)
    wvP = nc.declare_dram_parameter("wvP", [128, HID], bf16, isOutput=False)
    woP = nc.declare_dram_parameter("woP", [128, 8 * HID], bf16,
                                    isOutput=False)
    bqd = nc.declare_dram_parameter("bq", [128, 1], f32, isOutput=False)
    bkd = nc.declare_dram_parameter("bk", [128, 1], f32, isOutput=False)
    bod = nc.declare_dram_parameter("bo2", [128, 8], f32, isOutput=False)
    cosd = nc.declare_dram_parameter("cosT", [128, SB], bf16, isOutput=False)
    sind = nc.declare_dram_parameter("sinS", [128, SB], bf16, isOutput=False)
    out_ext = nc.declare_dram_parameter("out", [HID, RB], bf16, isOutput=True)

    a2a_in = nc.dram_tensor("a2a_in", [NC, OSL, RB], bf16)
    a2a_out = nc.dram_tensor("a2a_out", [NC, OSL, RB], bf16)

    NHC = HID // 128  # 8 hidden chunks
    NSG = SB // 512   # 8 seq groups

    with tile.TileContext(nc) as tc:
        with (
            tc.tile_pool(name="persist", bufs=1) as pp,
            tc.tile_pool(name="xs", bufs=9) as xp,
            tc.tile_pool(name="work", bufs=2) as wp,
            tc.tile_pool(name="exp", bufs=2) as ep,
        ):
            # ---------- consts ----------
            def pload(dram_ap, shape, dt_, tag):
                t = pp.tile(shape, dt_, tag=tag, name=tag)
                nc.sync.dma_start(out=t[:, :], in_=dram_ap)
                return t

            wq_sb = pload(wqP[:, :], [128, HID], bf16, "wq_sb")
            wk_sb = pload(wkP[:, :], [128, HID], bf16, "wk_sb")
            wv_sb = pload(wvP[:, :], [128, HID], bf16, "wv_sb")
            wqb = [wq_sb[:, 128 * c:128 * (c + 1)] for c in range(NHC)]
            wkb = [wk_sb[:, 128 * c:128 * (c + 1)] for c in range(NHC)]
            wvb = [wv_sb[:, 128 * c:128 * (c + 1)] for c in range(NHC)]
            bq_sb = pload(bqd[:, :], [128, 1], f32, "bq")
            bk_sb = pload(bkd[:, :], [128, 1], f32, "bk")
            bo_sb = pload(bod[:, :], [128, 8], f32, "bo")
            # cos/sin are not needed until the first rope quarter: load them
            # in column halves behind the first projection matmul loads.
            cos_sb = pp.tile([128, SB], bf16, tag="cos")
            sin_sb = pp.tile([128, SB], bf16, tag="sin")

            # PSUM pools for phases 1-4 (8 banks exactly); closed before the
            # output projection. "mm1024" serves qk-proj psums then the score
            # tiles; acc0-3 serve v-proj psums then the ctx accumulators.
            _cmA = tc.tile_pool(name="psA", bufs=2, space="PSUM")
            _cmB = tc.tile_pool(name="psB", bufs=1, space="PSUM")
            psA = _cmA.__enter__()
            psB = _cmB.__enter__()

            # ---------- phase 1: QKV projections (x streamed) + fused RoPE --
            # qsb/ksb: pre-rope projection output; qpk/kpk: post-rope, packed
            # head0 rows 0-63 / head1 rows 64-127 for 64-row PE tiling.
            qsb = wp.tile([128, SB], bf16, tag="qsb", bufs=1)
            ksb = wp.tile([128, SB], bf16, tag="ksb", bufs=1)
            # per-head rope outputs, d zero-padded to 128 partitions: K=128
            # score matmuls keep the full PE array streaming continuously
            # (sustained p-state) with no tile-mode switches anywhere.
            qrh = [pp.tile([128, SB], bf16, tag=f"qrh{h}", name=f"qrh{h}")
                   for h in range(HPC)]
            krh = [pp.tile([128, SB], bf16, tag=f"krh{h}", name=f"krh{h}")
                   for h in range(HPC)]
            for t in qrh + krh:
                nc.gpsimd.memset(t[64:128, :], 0.0)

            def rope_quarter(src, dsts, q4):
                sl = slice(1024 * q4, 1024 * (q4 + 1))
                qswp = wp.tile([128, 1024], bf16, tag="qswp")
                for blk in range(4):
                    dlo = 32 * blk
                    srow = 32 * (blk + 1) if blk % 2 == 0 else 32 * (blk - 1)
                    nc.sync.dma_start(
                        out=qswp[dlo:dlo + 32, :],
                        in_=src[srow:srow + 32, sl])
                t1 = wp.tile([128, 1024], bf16, tag="ropet1")
                t2 = wp.tile([128, 1024], bf16, tag="ropet2")
                rt = wp.tile([128, 1024], bf16, tag="ropert")
                nc.vector.tensor_mul(t1[:, :], src[:, sl], cos_sb[:, sl])
                nc.vector.tensor_mul(t2[:, :], qswp[:, :], sin_sb[:, sl])
                nc.vector.tensor_add(rt[:, :], t1[:, :], t2[:, :])
                for h in range(HPC):
                    nc.sync.dma_start(
                        out=dsts[h][0:64, sl],
                        in_=rt[64 * h:64 * (h + 1), :])

            # v tiles: [v0(64) | ones | v1(64) | ones] so each head's lhsT
            # slice ([0:65] / [65:130]) puts the softmax denominator at
            # output partition 64.
            vsb = [None] * 32
            xb2 = {}

            def proj_sg(sg):
                # q and k projections share one [128,1024] psum (q cols
                # 0:512, k cols 512:1024; the 1/8 q-scale is folded into Wq
                # host-side) so no psum tile ever spans an attention pass.
                bh = sg // 4
                if sg % 4 == 0:
                    for c in range(NHC):
                        t = xp.tile([128, 2048], bf16, tag="xb")
                        nc.sync.dma_start(
                            out=t[:, :],
                            in_=xT[128 * c:128 * (c + 1),
                                   2048 * bh:2048 * (bh + 1)])
                        xb2[c] = t
                xbt = [xb2[c][:, 512 * (sg % 4):512 * (sg % 4 + 1)]
                       for c in range(NHC)]
                if sg == 0:
                    for qrt in range(4):
                        hs = slice(1024 * qrt, 1024 * (qrt + 1))
                        nc.sync.dma_start(out=cos_sb[:, hs],
                                          in_=cosd[:, hs])
                        nc.sync.dma_start(out=sin_sb[:, hs],
                                          in_=sind[:, hs])
                ps = psA.tile([128, 1024], f32, tag="mm1024", name="qkps")
                for off, wb in ((0, wqb), (512, wkb)):
                    for c in range(NHC):
                        nc.tensor.matmul(
                            ps[:, off:off + 512],
                            lhsT=wb[c], rhs=xbt[c],
                            start=(c == 0), stop=(c == NHC - 1))
                sl5 = slice(512 * sg, 512 * (sg + 1))
                nc.vector.tensor_scalar(
                    qsb[:, sl5], ps[:, 0:512], 1.0, bq_sb[:, 0:1],
                    mybir.AluOpType.mult, mybir.AluOpType.add)
                nc.vector.tensor_scalar(
                    ksb[:, sl5], ps[:, 512:1024], 1.0, bk_sb[:, 0:1],
                    mybir.AluOpType.mult, mybir.AluOpType.add)
                for st4 in range(4):
                    st = 4 * sg + st4
                    ps = psB.tile([128, OSL], f32, tag=f"acc{st4 % 2}",
                                  padded_shape=[128, 512], bufs=2)
                    for c in range(NHC):
                        nc.tensor.matmul(
                            ps[:, :],
                            lhsT=xb2[c][:, 512 * (sg % 4) + 128 * st4:
                                        512 * (sg % 4) + 128 * (st4 + 1)],
                            rhs=wvb[c],
                            start=(c == 0), stop=(c == NHC - 1))
                    vt = pp.tile([128, 130], bf16,
                                 tag=f"vsb{st}", name=f"vsb{st}")
                    nc.gpsimd.memset(vt[:, 64:65], 1.0)
                    nc.gpsimd.memset(vt[:, 129:130], 1.0)
                    nc.scalar.copy(vt[:, 0:64], ps[:, 0:64])
                    nc.scalar.copy(vt[:, 65:129], ps[:, 64:128])
                    vsb[st] = vt
                if sg % 2 == 1:
                    rope_quarter(qsb, qrh, sg // 2)
                    rope_quarter(ksb, krh, sg // 2)

            for sg in range(4):
                proj_sg(sg)

            # Wo chunks: needed only in phase 5, but loaded here so the DMA
            # hides under attention.
            wo_sb = pp.tile([128, 8 * HID], bf16, tag="wo_sb",
                            name="wo_sb")
            nc.sync.dma_start(out=wo_sb[:, :], in_=woP[:, :])
            wob = [wo_sb[:, HID * c:HID * (c + 1)] for c in range(NHC)]

            # ---------- phase 3: attention, 64-row PE array tiling ----------
            # Per ks: head0 runs on array tile (0,0), head1 on (64,0); the
            # ctx contraction (K=128 kpos) is split into two 64-row halves
            # with separate accumulators, merged on DVE at pass end.
            last_nrm = [None]

            def attn_pass(b, qs):
                    q0 = S * b + 512 * qs
                    accs = [psB.tile([65, 512], f32, tag=f"acc{i}",
                                     padded_shape=[128, 512], bufs=2,
                                     name=f"ctxacc{i}")
                            for i in range(HPC)]
                    pend = []    # deferred ctx emission (2-deep pipeline)

                    def ctx_mm(ks, et):
                        vt = vsb[(S * b) // 128 + ks]
                        st = ks == 0
                        sp = ks == 15
                        for h in range(HPC):
                            c0 = 65 * h
                            nc.tensor.matmul(
                                accs[h][:, :],
                                lhsT=vt[:, c0:c0 + 65],
                                rhs=et[:, 512 * h:512 * (h + 1)],
                                start=st, stop=sp)

                    for ks in range(16):
                        k0 = S * b + 128 * ks
                        sps = psA.tile([128, 1024], f32, tag="mm1024")
                        nc.tensor.matmul(
                            sps[:, 0:512], lhsT=krh[0][:, k0:k0 + 128],
                            rhs=qrh[0][:, q0:q0 + 512],
                            start=True, stop=True)
                        nc.tensor.matmul(
                            sps[:, 512:1024], lhsT=krh[1][:, k0:k0 + 128],
                            rhs=qrh[1][:, q0:q0 + 512],
                            start=True, stop=True)
                        et = ep.tile([128, 1024], bf16, tag="expT", bufs=4)
                        if ks in DVE_KS:
                            mid = ep.tile([128, 1024], f32, tag="expM",
                                          bufs=2)
                            nc.vector._custom_dve(
                                EXP_A, out=mid[:, :], in0=sps[:, :],
                                s0=1.0 / EXP_N)
                            nc.vector._custom_dve(
                                EXP_B, out=et[:, :], in0=mid[:, :])
                        else:
                            nc.scalar.activation(et[:, :], sps[:, :], AF.Exp)
                        pend.append((ks, et))
                        if len(pend) > 2:
                            ctx_mm(*pend.pop(0))
                    for p_ in pend:
                        ctx_mm(*p_)

                    rbs = []
                    for h in range(HPC):
                        # denominator row to SBUF, reshape to [128,4] so the
                        # reciprocal runs 128 lanes wide, reshape back,
                        # broadcast on gpsimd.
                        rs1 = ep.tile([65, 512], f32, tag="rs1", bufs=2)
                        nc.vector.tensor_copy(
                            rs1[64:65, :], accs[h][64:65, :])
                        rsP = ep.tile([128, 4], f32, tag="rsP", bufs=2)
                        nc.sync.dma_start(out=rsP[:, :], in_=rs1[64:65, :])
                        rPr = ep.tile([128, 4], f32, tag="rPr", bufs=2)
                        nc.vector.reciprocal(rPr[:, :], rsP[:, :])
                        rc0 = ep.tile([1, 512], f32, tag="rc0", bufs=2)
                        nc.sync.dma_start(out=rc0[:, :], in_=rPr[:, :])
                        rb = ep.tile([64, 512], f32, tag="rb", bufs=2)
                        nc.gpsimd.partition_broadcast(rb[:, :], rc0[:, :])
                        rbs.append(rb)
                    for h in range(HPC):
                        nrm = ep.tile([64, 512], bf16, tag="nrm", bufs=2)
                        nc.vector.tensor_mul(
                            nrm[:, :], accs[h][0:64, :], rbs[h][:, :])
                        nc.sync.dma_start(
                            out=a2a_in[4 * b + qs, 64 * h:64 * (h + 1), :],
                            in_=nrm[:, :])
                        last_nrm[0] = nrm

            # batch-1 projections hide in the exp-gated gaps of the first
            # batch-0 passes (the ScalarE/DVE exp wall runs continuously).
            attn_pass(0, 0)
            proj_sg(4)
            proj_sg(5)
            attn_pass(0, 1)
            proj_sg(6)
            proj_sg(7)
            attn_pass(0, 2)
            attn_pass(0, 3)
            for qs in range(4):
                attn_pass(1, qs)

            # ---------- phase 4: AllToAll ----------
            nc.gpsimd.collective_compute(
                "AllToAll", mybir.AluOpType.bypass,
                replica_groups=[list(range(NC))],
                ins=[a2a_in.ap().opt()],
                outs=[a2a_out.ap().opt()])

            # ---------- phase 5: output projection ----------
            _cmB.__exit__(None, None, None)
            _cmA.__exit__(None, None, None)
            _cmO = tc.tile_pool(name="psO", bufs=1, space="PSUM")
            psO = _cmO.__enter__()

            # Keep the PE array p-state up across the AllToAll wait: a chain
            # of matmuls anchored on the last normalized ctx tile so they
            # cannot run before attention finishes.
            dumsrc = pp.tile([128, 512], bf16, tag="dumsrc")
            nc.gpsimd.memset(dumsrc[:, :], 0.0)
            nc.scalar.copy(dumsrc[0:64, :], last_nrm[0][:, :])
            dum = psO.tile([128, 512], f32, tag="dum", bufs=1)
            for i in range(DUM):
                nc.tensor.matmul(
                    dum[:, :], lhsT=wob[0][:, 0:128], rhs=dumsrc[:, :],
                    start=True, stop=True)
            dumr = ep.tile([128, 512], f32, tag="dumr")
            nc.vector.tensor_copy(dumr[:, :], dum[:, :])
            dead = nc.dram_tensor("dead", [128, 512], f32)
            nc.sync.dma_start(out=dead[:, :], in_=dumr[:, :])
            # Load all 8 received o-chunks first (1MB total), then run the
            # accumulation ot-outer so each out-tile finishes early and its
            # bias-add + store overlap the remaining matmuls.
            cxs = []
            for c in range(NHC):
                cx = pp.tile([128, RB], bf16, tag=f"cxb{c}", name=f"cxb{c}")
                nc.sync.dma_start(out=cx[:, :], in_=a2a_out[c, :, :])
                cxs.append(cx)
            for ot in range(8):
                ops = psO.tile([128, 512], f32, tag="ops", bufs=4)
                for c in range(NHC):
                    nc.tensor.matmul(
                        ops[:, :],
                        lhsT=wob[c][:, 128 * ot:128 * (ot + 1)],
                        rhs=cxs[c][:, :],
                        start=(c == 0), stop=(c == NHC - 1))
                osb = ep.tile([128, RB], bf16, tag="osb", bufs=3)
                nc.scalar.activation(
                    osb[:, :], ops[:, :], AF.Identity,
                    bias=bo_sb[:, ot:ot + 1], scale=1.0)
                nc.sync.dma_start(
                    out=out_ext[128 * ot:128 * (ot + 1), :], in_=osb[:, :])
            _cmO.__exit__(None, None, None)

    nc.finalize()
    return nc


def _host_tables():
    inv = 1.0 / (ROPE_BASE ** (np.arange(0, D, 2, dtype=np.float64) / D))
    pos = np.arange(S, dtype=np.float64)
    freqs = np.outer(pos, inv)                      # [S, 32]
    emb = np.concatenate([freqs, freqs], axis=-1)   # [S, 64]
    cosT = np.cos(emb).T.astype(np.float32)         # [64, S]
    sinT = np.sin(emb).T.astype(np.float32)
    sinS = np.concatenate([-sinT[:32], sinT[32:]], axis=0)
    cos2 = np.ascontiguousarray(np.tile(cosT, (2, 2)))   # [128, 2S]
    sin2 = np.ascontiguousarray(np.tile(sinS, (2, 2)))
    return cos2, sin2


def kernel(**inputs):
    import ml_dtypes
    from concourse.bass_utils import run_bass_kernel_spmd

    global _cached, _last_in_maps
    if _cached is None:
        _cached = _build_nc()
    nc = _cached

    bf = ml_dtypes.bfloat16
    hs = np.asarray(inputs["hidden_states"], dtype=np.float32)
    Wq = np.asarray(inputs["Wq"], dtype=np.float32)
    bq = np.asarray(inputs["bq"], dtype=np.float32)
    Wk = np.asarray(inputs["Wk"], dtype=np.float32)
    bk = np.asarray(inputs["bk"], dtype=np.float32)
    Wv = np.asarray(inputs["Wv"], dtype=np.float32)
    bv = np.asarray(inputs["bv"], dtype=np.float32)
    Wo = np.asarray(inputs["Wo"], dtype=np.float32)
    bo = np.asarray(inputs["bo"], dtype=np.float32)

    cos2, sin2 = _host_tables()
    cos2 = cos2.astype(bf)
    sin2 = sin2.astype(bf)
    bo2 = bo + bv @ Wo.T                                 # fold v-bias exactly
    bo2m = np.ascontiguousarray(bo2.reshape(8, 128).T)   # [128, 8]
    xTfull = np.ascontiguousarray(
        np.concatenate([hs[0].T, hs[1].T], axis=1)).astype(bf)  # [1024, 4096]

    def pack_w(A, width):
        # [1024, width] -> sbuf layout [128, 8*width]: chunk c of 128 rows
        # lands at columns [width*c, width*(c+1))
        return np.ascontiguousarray(
            A.reshape(8, 128, width).transpose(1, 0, 2).reshape(128, -1))

    woPc = pack_w(Wo.T, HID).astype(bf)  # [128, 8192]

    in_maps = []
    for c in range(NC):
        sl = slice(OSL * c, OSL * (c + 1))
        in_maps.append({
            "xT": xTfull,
            "wqP": pack_w(Wq[sl, :].T * 0.125, OSL).astype(bf),
            "wkP": pack_w(Wk[sl, :].T, OSL).astype(bf),
            "wvP": pack_w(Wv[sl, :].T, OSL).astype(bf),
            "woP": woPc,
            "bq": np.ascontiguousarray((bq[sl] * 0.125)[:, None]),
            "bk": np.ascontiguousarray(bk[sl][:, None]),
            "bo2": bo2m,
            "cosT": cos2,
            "sinS": sin2,
        })

    _last_in_maps = in_maps
    res = run_bass_kernel_spmd(nc, in_maps, core_ids=list(range(NC)))
    out = np.empty((2, S, HID), dtype=np.float32)
    for c in range(NC):
        b, g = divmod(c, 4)
        out[b, RB * g:RB * (g + 1), :] = res.results[c]["out"].T.astype(np.float32)
    return out


# revision 37
# speedup vs baseline: 1.0221x; 1.0221x over previous
"""AttentionWithRoPE distributed Trainium2 kernel (8 NeuronCores).

Sharding: pure 8-way tensor parallel over heads (2 heads = 128 hidden cols
per core), both batches on every core (seq concatenated to 4096 cols).
Everything stays transposed ([feature, seq] layouts) so no on-device
transposes are needed anywhere:
  - QKV projections consume xT (host-transposed, batch-concat, bf16) as the
    moving operand, streamed in [128, 512] chunks.
  - RoPE applied on packed qT/kT [128, s] tiles (head0 rows 0-63, head1 rows
    64-127); the 32-row half-rotation is done with sbuf->sbuf DMAs, with a
    sign-folded sin table so no negation op is needed; all ops bf16.
  - scores: 64-row PE array tiling -- head0 on tile (0,0), head1 on (64,0),
    K=64 each, running concurrently in the two array halves.
  - exp split across ScalarE (native Exp) and VectorE (custom 2-pass DVE op
    computing (1+x/16384)^16384, rel err <=0.11% for |x|<=6).
  - ctx matmuls split per 64-row kpos half into 4 PSUM accumulators (merged
    on DVE); v tiles carry a ones-column per head so the softmax denominator
    falls out of the matmul at output partition 64.
  - normalization: reciprocal on DVE, partition_broadcast + multiply on
    GpSimd (all-bf16), shipped straight to the AllToAll buffer.
  - AllToAll (bf16, all 8 cores) exchanges 512-row blocks of ctx^T.
  - Output projection with full Wo produces out^T [1024, 512] for this
    core's 512 global rows; host transposes back (free).
Bias folds (host side): v-bias folds into the output bias exactly (softmax
rows sum to 1); q is pre-scaled by 1/sqrt(64) inside its bias-copy.
Compute dtype bf16 (fp32 PSUM accumulation).
"""

import numpy as np

HID = 1024
S = 2048
SB = 2 * S       # both batches, seq-concatenated
NHEAD = 16
D = 64
HPC = 2          # heads per core
OSL = 128        # hidden slice per core (HPC * D)
RB = 512         # global row block per core after AllToAll
NC = 8
ROPE_BASE = 10000.0

EXP_N = 16384.0  # (1+x/N)^N exp approx on DVE
DVE_KS = (2, 5, 8, 11)   # ks tiles exp'd on DVE instead of ScalarE
DUM = 155        # dummy matmuls keeping the PE p-state up across the a2a

_cached = None
_last_in_maps = None


def _register_exp_ops():
    from concourse import dve_ops as DO
    from concourse.dve_spec import (
        Spec, Src0, C0, One, sq, lower, _has_src1 as has_src1)
    from concourse.dve_uop import DveOpSpec
    from concourse.dve_table_gen import dve_ver_for

    have = {op.name: op for op in DO.OPS}
    if "EXP_POW_A" in have:
        return have["EXP_POW_A"], have["EXP_POW_B"]

    b = Src0 * C0 + One
    for _ in range(6):
        b = sq(b)
    specA = Spec(body=b, reference=lambda in0, in1, s0, s1, imm2:
                 (1.0 + in0 * s0) ** 64)
    c = Src0
    for _ in range(8):
        c = sq(c)
    specB = Spec(body=c, reference=lambda in0, in1, s0, s1, imm2: in0 ** 256)

    out = []
    for name, spec in (("EXP_POW_A", specA), ("EXP_POW_B", specB)):
        DO.OPS.append(DO.DveOp(name, spec, subdim=False, uops_sha={}))
        opcode = DO._CUSTOM_DVE_ROW_BASE + len(DO.OPS) - 1
        DO._SUB_OPCODE_FOR_NAME[name] = opcode
        shas = {}
        for ver in ("v3", "v4"):
            try:
                s = DveOpSpec(name=name, opcode=opcode,
                              uops=lower(spec, ver=ver),
                              rd1_en=has_src1(spec))
                shas[ver] = s.sha(ver)
            except Exception:
                pass
        op = DO.DveOp(name, spec, subdim=False, uops_sha=shas)
        DO.OPS[-1] = op
        DO.CUSTOM_DVE_SPECS[name] = spec
        out.append(op)
    return out[0], out[1]


def _build_nc():
    import concourse.bacc as bacc
    import concourse.mybir as mybir
    from concourse import tile

    EXP_A, EXP_B = _register_exp_ops()

    f32 = mybir.dt.float32
    bf16 = mybir.dt.bfloat16
    AF = mybir.ActivationFunctionType

    nc = bacc.Bacc(None, target_bir_lowering=False)

    xT = nc.declare_dram_parameter("xT", [HID, SB], bf16, isOutput=False)
    wqP = nc.declare_dram_parameter("wqP", [128, HID], bf16, isOutput=False)
    wkP = nc.declare_dram_parameter("wkP", [128, HID], bf16, isOutput=False)
    wvP = nc.declare_dram_parameter("wvP", [128, HID], bf16, isOutput=False)
    woP = nc.declare_dram_parameter("woP", [128, 8 * HID], bf16,
                                    isOutput=False)
    bqd = nc.declare_dram_parameter("bq", [128, 1], f32, isOutput=False)
    bkd = nc.declare_dram_parameter("bk", [128, 1], f32, isOutput=False)
    bod = nc.declare_dram_parameter("bo2", [128, 8], f32, isOutput=False)
    cosd = nc.declare_dram_parameter("cosT", [128, SB], bf16, isOutput=False)
    sind = nc.declare_dram_parameter("sinS", [128, SB], bf16, isOutput=False)
    out_ext = nc.declare_dram_parameter("out", [HID, RB], bf16, isOutput=True)

    a2a_in = nc.dram_tensor("a2a_in", [NC, OSL, RB], bf16)
    a2a_out = nc.dram_tensor("a2a_out", [NC, OSL, RB], bf16)

    NHC = HID // 128  # 8 hidden chunks
    NSG = SB // 512   # 8 seq groups

    with tile.TileContext(nc) as tc:
        with (
            tc.tile_pool(name="persist", bufs=1) as pp,
            tc.tile_pool(name="xs", bufs=9) as xp,
            tc.tile_pool(name="work", bufs=2) as wp,
            tc.tile_pool(name="exp", bufs=2) as ep,
        ):
            # ---------- consts ----------
            def pload(dram_ap, shape, dt_, tag):
                t = pp.tile(shape, dt_, tag=tag, name=tag)
                nc.sync.dma_start(out=t[:, :], in_=dram_ap)
                return t

            wq_sb = pload(wqP[:, :], [128, HID], bf16, "wq_sb")
            wk_sb = pload(wkP[:, :], [128, HID], bf16, "wk_sb")
            wv_sb = pload(wvP[:, :], [128, HID], bf16, "wv_sb")
            wqb = [wq_sb[:, 128 * c:128 * (c + 1)] for c in range(NHC)]
            wkb = [wk_sb[:, 128 * c:128 * (c + 1)] for c in range(NHC)]
            wvb = [wv_sb[:, 128 * c:128 * (c + 1)] for c in range(NHC)]
            bq_sb = pload(bqd[:, :], [128, 1], f32, "bq")
            bk_sb = pload(bkd[:, :], [128, 1], f32, "bk")
            bo_sb = pload(bod[:, :], [128, 8], f32, "bo")
            # cos/sin are not needed until the first rope quarter: load them
            # in column halves behind the first projection matmul loads.
            cos_sb = pp.tile([128, SB], bf16, tag="cos")
            sin_sb = pp.tile([128, SB], bf16, tag="sin")

            # PSUM pools for phases 1-4 (8 banks exactly); closed before the
            # output projection. "mm1024" serves qk-proj psums then the score
            # tiles; acc0-3 serve v-proj psums then the ctx accumulators.
            _cmA = tc.tile_pool(name="psA", bufs=2, space="PSUM")
            _cmB = tc.tile_pool(name="psB", bufs=1, space="PSUM")
            psA = _cmA.__enter__()
            psB = _cmB.__enter__()

            # ---------- phase 1: QKV projections (x streamed) + fused RoPE --
            # qsb/ksb: pre-rope projection output; qpk/kpk: post-rope, packed
            # head0 rows 0-63 / head1 rows 64-127 for 64-row PE tiling.
            qsb = wp.tile([128, SB], bf16, tag="qsb", bufs=1)
            ksb = wp.tile([128, SB], bf16, tag="ksb", bufs=1)
            # per-head rope outputs, d zero-padded to 128 partitions: K=128
            # score matmuls keep the full PE array streaming continuously
            # (sustained p-state) with no tile-mode switches anywhere.
            qrh = [pp.tile([128, SB], bf16, tag=f"qrh{h}", name=f"qrh{h}")
                   for h in range(HPC)]
            krh = [pp.tile([128, SB], bf16, tag=f"krh{h}", name=f"krh{h}")
                   for h in range(HPC)]
            for t in qrh + krh:
                nc.gpsimd.memset(t[64:128, :], 0.0)

            def rope_quarter(src, dsts, q4):
                sl = slice(1024 * q4, 1024 * (q4 + 1))
                qswp = wp.tile([128, 1024], bf16, tag="qswp")
                for blk in range(4):
                    dlo = 32 * blk
                    srow = 32 * (blk + 1) if blk % 2 == 0 else 32 * (blk - 1)
                    nc.sync.dma_start(
                        out=qswp[dlo:dlo + 32, :],
                        in_=src[srow:srow + 32, sl])
                t1 = wp.tile([128, 1024], bf16, tag="ropet1")
                t2 = wp.tile([128, 1024], bf16, tag="ropet2")
                rt = wp.tile([128, 1024], bf16, tag="ropert")
                nc.vector.tensor_mul(t1[:, :], src[:, sl], cos_sb[:, sl])
                nc.vector.tensor_mul(t2[:, :], qswp[:, :], sin_sb[:, sl])
                nc.vector.tensor_add(rt[:, :], t1[:, :], t2[:, :])
                for h in range(HPC):
                    nc.sync.dma_start(
                        out=dsts[h][0:64, sl],
                        in_=rt[64 * h:64 * (h + 1), :])

            # v tiles: [v0(64) | ones | v1(64) | ones] so each head's lhsT
            # slice ([0:65] / [65:130]) puts the softmax denominator at
            # output partition 64.
            vsb = [None] * 32
            xb2 = {}

            def proj_sg(sg):
                # q and k projections share one [128,1024] psum (q cols
                # 0:512, k cols 512:1024; the 1/8 q-scale is folded into Wq
                # host-side) so no psum tile ever spans an attention pass.
                bh = sg // 4
                if sg % 4 == 0:
                    for c in range(NHC):
                        t = xp.tile([128, 2048], bf16, tag="xb")
                        nc.sync.dma_start(
                            out=t[:, :],
                            in_=xT[128 * c:128 * (c + 1),
                                   2048 * bh:2048 * (bh + 1)])
                        xb2[c] = t
                xbt = [xb2[c][:, 512 * (sg % 4):512 * (sg % 4 + 1)]
                       for c in range(NHC)]
                if sg == 0:
                    for qrt in range(4):
                        hs = slice(1024 * qrt, 1024 * (qrt + 1))
                        nc.sync.dma_start(out=cos_sb[:, hs],
                                          in_=cosd[:, hs])
                        nc.sync.dma_start(out=sin_sb[:, hs],
                                          in_=sind[:, hs])
                ps = psA.tile([128, 1024], f32, tag="mm1024", name="qkps")
                for off, wb in ((0, wqb), (512, wkb)):
                    for c in range(NHC):
                        nc.tensor.matmul(
                            ps[:, off:off + 512],
                            lhsT=wb[c], rhs=xbt[c],
                            start=(c == 0), stop=(c == NHC - 1))
                sl5 = slice(512 * sg, 512 * (sg + 1))
                nc.vector.tensor_scalar(
                    qsb[:, sl5], ps[:, 0:512], 1.0, bq_sb[:, 0:1],
                    mybir.AluOpType.mult, mybir.AluOpType.add)
                nc.vector.tensor_scalar(
                    ksb[:, sl5], ps[:, 512:1024], 1.0, bk_sb[:, 0:1],
                    mybir.AluOpType.mult, mybir.AluOpType.add)
                for st4 in range(4):
                    st = 4 * sg + st4
                    ps = psB.tile([128, OSL], f32, tag=f"acc{st4 % 2}",
                                  padded_shape=[128, 512], bufs=2)
                    for c in range(NHC):
                        nc.tensor.matmul(
                            ps[:, :],
                            lhsT=xb2[c][:, 512 * (sg % 4) + 128 * st4:
                                        512 * (sg % 4) + 128 * (st4 + 1)],
                            rhs=wvb[c],
                            start=(c == 0), stop=(c == NHC - 1))
                    vt = pp.tile([128, 130], bf16,
                                 tag=f"vsb{st}", name=f"vsb{st}")
                    nc.gpsimd.memset(vt[:, 64:65], 1.0)
                    nc.gpsimd.memset(vt[:, 129:130], 1.0)
                    nc.scalar.copy(vt[:, 0:64], ps[:, 0:64])
                    nc.scalar.copy(vt[:, 65:129], ps[:, 64:128])
                    vsb[st] = vt
                if sg % 2 == 1:
                    rope_quarter(qsb, qrh, sg // 2)
                    rope_quarter(ksb, krh, sg // 2)

            for sg in range(4):
                proj_sg(sg)

            # Wo chunks: needed only in phase 5, but loaded here so the DMA
            # hides under attention.
            wo_sb = pp.tile([128, 8 * HID], bf16, tag="wo_sb",
                            name="wo_sb")
            nc.sync.dma_start(out=wo_sb[:, :], in_=woP[:, :])
            wob = [wo_sb[:, HID * c:HID * (c + 1)] for c in range(NHC)]

            # ---------- phase 3: attention, 64-row PE array tiling ----------
            # Per ks: head0 runs on array tile (0,0), head1 on (64,0); the
            # ctx contraction (K=128 kpos) is split into two 64-row halves
            # with separate accumulators, merged on DVE at pass end.
            last_nrm = [None]

            def attn_pass(b, qs):
                    q0 = S * b + 512 * qs
                    accs = [psB.tile([65, 512], f32, tag=f"acc{i}",
                                     padded_shape=[128, 512], bufs=2,
                                     name=f"ctxacc{i}")
                            for i in range(HPC)]
                    pend = []    # deferred ctx emission (2-deep pipeline)

                    def ctx_mm(ks, et):
                        vt = vsb[(S * b) // 128 + ks]
                        st = ks == 0
                        sp = ks == 15
                        for h in range(HPC):
                            c0 = 65 * h
                            nc.tensor.matmul(
                                accs[h][:, :],
                                lhsT=vt[:, c0:c0 + 65],
                                rhs=et[:, 512 * h:512 * (h + 1)],
                                start=st, stop=sp)

                    for ks in range(16):
                        k0 = S * b + 128 * ks
                        sps = psA.tile([128, 1024], f32, tag="mm1024")
                        nc.tensor.matmul(
                            sps[:, 0:512], lhsT=krh[0][:, k0:k0 + 128],
                            rhs=qrh[0][:, q0:q0 + 512],
                            start=True, stop=True)
                        nc.tensor.matmul(
                            sps[:, 512:1024], lhsT=krh[1][:, k0:k0 + 128],
                            rhs=qrh[1][:, q0:q0 + 512],
                            start=True, stop=True)
                        et = ep.tile([128, 1024], bf16, tag="expT", bufs=6)
                        if ks in DVE_KS:
                            mid = ep.tile([128, 1024], f32, tag="expM",
                                          bufs=2)
                            nc.vector._custom_dve(
                                EXP_A, out=mid[:, :], in0=sps[:, :],
                                s0=1.0 / EXP_N)
                            nc.vector._custom_dve(
                                EXP_B, out=et[:, :], in0=mid[:, :])
                        else:
                            nc.scalar.activation(et[:, :], sps[:, :], AF.Exp)
                        pend.append((ks, et))
                        if len(pend) > 2:
                            ctx_mm(*pend.pop(0))
                    for p_ in pend:
                        ctx_mm(*p_)

                    rbs = []
                    for h in range(HPC):
                        # denominator row to SBUF, reshape to [128,4] so the
                        # reciprocal runs 128 lanes wide, reshape back,
                        # broadcast on gpsimd.
                        rs1 = ep.tile([65, 512], f32, tag="rs1", bufs=2)
                        nc.vector.tensor_copy(
                            rs1[64:65, :], accs[h][64:65, :])
                        rsP = ep.tile([128, 4], f32, tag="rsP", bufs=2)
                        nc.sync.dma_start(out=rsP[:, :], in_=rs1[64:65, :])
                        rPr = ep.tile([128, 4], f32, tag="rPr", bufs=2)
                        nc.vector.reciprocal(rPr[:, :], rsP[:, :])
                        rc0 = ep.tile([1, 512], f32, tag="rc0", bufs=2)
                        nc.sync.dma_start(out=rc0[:, :], in_=rPr[:, :])
                        rb = ep.tile([64, 512], f32, tag="rb", bufs=2)
                        nc.gpsimd.partition_broadcast(rb[:, :], rc0[:, :])
                        rbs.append(rb)
                    for h in range(HPC):
                        nrm = ep.tile([64, 512], bf16, tag="nrm", bufs=2)
                        nc.vector.tensor_mul(
                            nrm[:, :], accs[h][0:64, :], rbs[h][:, :])
                        nc.sync.dma_start(
                            out=a2a_in[4 * b + qs, 64 * h:64 * (h + 1), :],
                            in_=nrm[:, :])
                        last_nrm[0] = nrm

            # batch-1 projections hide in the exp-gated gaps of the first
            # batch-0 passes (the ScalarE/DVE exp wall runs continuously).
            attn_pass(0, 0)
            proj_sg(4)
            proj_sg(5)
            attn_pass(0, 1)
            proj_sg(6)
            proj_sg(7)
            attn_pass(0, 2)
            attn_pass(0, 3)
            for qs in range(4):
                attn_pass(1, qs)

            # ---------- phase 4: AllToAll ----------
            nc.gpsimd.collective_compute(
                "AllToAll", mybir.AluOpType.bypass,
                replica_groups=[list(range(NC))],
                ins=[a2a_in.ap().opt()],
                outs=[a2a_out.ap().opt()])

            # ---------- phase 5: output projection ----------
            _cmB.__exit__(None, None, None)
            _cmA.__exit__(None, None, None)
            _cmO = tc.tile_pool(name="psO", bufs=1, space="PSUM")
            psO = _cmO.__enter__()

            # Keep the PE array p-state up across the AllToAll wait: a chain
            # of matmuls anchored on the last normalized ctx tile so they
            # cannot run before attention finishes.
            dumsrc = pp.tile([128, 512], bf16, tag="dumsrc")
            nc.gpsimd.memset(dumsrc[:, :], 0.0)
            nc.scalar.copy(dumsrc[0:64, :], last_nrm[0][:, :])
            dum = psO.tile([128, 512], f32, tag="dum", bufs=1)
            for i in range(DUM):
                nc.tensor.matmul(
                    dum[:, :], lhsT=wob[0][:, 0:128], rhs=dumsrc[:, :],
                    start=True, stop=True)
            dumr = ep.tile([128, 512], f32, tag="dumr")
            nc.vector.tensor_copy(dumr[:, :], dum[:, :])
            dead = nc.dram_tensor("dead", [128, 512], f32)
            nc.sync.dma_start(out=dead[:, :], in_=dumr[:, :])
            # Load all 8 received o-chunks first (1MB total), then run the
            # accumulation ot-outer so each out-tile finishes early and its
            # bias-add + store overlap the remaining matmuls.
            cxs = []
            for c in range(NHC):
                cx = pp.tile([128, RB], bf16, tag=f"cxb{c}", name=f"cxb{c}")
                nc.sync.dma_start(out=cx[:, :], in_=a2a_out[c, :, :])
                cxs.append(cx)
            for ot in range(8):
                ops = psO.tile([128, 512], f32, tag="ops", bufs=4)
                for c in range(NHC):
                    nc.tensor.matmul(
                        ops[:, :],
                        lhsT=wob[c][:, 128 * ot:128 * (ot + 1)],
                        rhs=cxs[c][:, :],
                        start=(c == 0), stop=(c == NHC - 1))
                osb = ep.tile([128, RB], bf16, tag="osb", bufs=3)
                nc.scalar.activation(
                    osb[:, :], ops[:, :], AF.Identity,
                    bias=bo_sb[:, ot:ot + 1], scale=1.0)
                nc.sync.dma_start(
                    out=out_ext[128 * ot:128 * (ot + 1), :], in_=osb[:, :])
            _cmO.__exit__(None, None, None)

    nc.finalize()
    return nc


def _host_tables():
    inv = 1.0 / (ROPE_BASE ** (np.arange(0, D, 2, dtype=np.float64) / D))
    pos = np.arange(S, dtype=np.float64)
    freqs = np.outer(pos, inv)                      # [S, 32]
    emb = np.concatenate([freqs, freqs], axis=-1)   # [S, 64]
    cosT = np.cos(emb).T.astype(np.float32)         # [64, S]
    sinT = np.sin(emb).T.astype(np.float32)
    sinS = np.concatenate([-sinT[:32], sinT[32:]], axis=0)
    cos2 = np.ascontiguousarray(np.tile(cosT, (2, 2)))   # [128, 2S]
    sin2 = np.ascontiguousarray(np.tile(sinS, (2, 2)))
    return cos2, sin2


def kernel(**inputs):
    import ml_dtypes
    from concourse.bass_utils import run_bass_kernel_spmd

    global _cached, _last_in_maps
    if _cached is None:
        _cached = _build_nc()
    nc = _cached

    bf = ml_dtypes.bfloat16
    hs = np.asarray(inputs["hidden_states"], dtype=np.float32)
    Wq = np.asarray(inputs["Wq"], dtype=np.float32)
    bq = np.asarray(inputs["bq"], dtype=np.float32)
    Wk = np.asarray(inputs["Wk"], dtype=np.float32)
    bk = np.asarray(inputs["bk"], dtype=np.float32)
    Wv = np.asarray(inputs["Wv"], dtype=np.float32)
    bv = np.asarray(inputs["bv"], dtype=np.float32)
    Wo = np.asarray(inputs["Wo"], dtype=np.float32)
    bo = np.asarray(inputs["bo"], dtype=np.float32)

    cos2, sin2 = _host_tables()
    cos2 = cos2.astype(bf)
    sin2 = sin2.astype(bf)
    bo2 = bo + bv @ Wo.T                                 # fold v-bias exactly
    bo2m = np.ascontiguousarray(bo2.reshape(8, 128).T)   # [128, 8]
    xTfull = np.ascontiguousarray(
        np.concatenate([hs[0].T, hs[1].T], axis=1)).astype(bf)  # [1024, 4096]

    def pack_w(A, width):
        # [1024, width] -> sbuf layout [128, 8*width]: chunk c of 128 rows
        # lands at columns [width*c, width*(c+1))
        return np.ascontiguousarray(
            A.reshape(8, 128, width).transpose(1, 0, 2).reshape(128, -1))

    woPc = pack_w(Wo.T, HID).astype(bf)  # [128, 8192]

    in_maps = []
    for c in range(NC):
        sl = slice(OSL * c, OSL * (c + 1))
        in_maps.append({
            "xT": xTfull,
            "wqP": pack_w(Wq[sl, :].T * 0.125, OSL).astype(bf),
            "wkP": pack_w(Wk[sl, :].T, OSL).astype(bf),
            "wvP": pack_w(Wv[sl, :].T, OSL).astype(bf),
            "woP": woPc,
            "bq": np.ascontiguousarray((bq[sl] * 0.125)[:, None]),
            "bk": np.ascontiguousarray(bk[sl][:, None]),
            "bo2": bo2m,
            "cosT": cos2,
            "sinS": sin2,
        })

    _last_in_maps = in_maps
    res = run_bass_kernel_spmd(nc, in_maps, core_ids=list(range(NC)))
    out = np.empty((2, S, HID), dtype=np.float32)
    for c in range(NC):
        b, g = divmod(c, 4)
        out[b, RB * g:RB * (g + 1), :] = res.results[c]["out"].T.astype(np.float32)
    return out
